# revision 10
# baseline (speedup 1.0000x reference)
"""MinusAttention kernel for Trainium2 (8 NeuronCores, Bass/Tile).

Math: score[i,j] = (w.q_i - w.k_j + b) / sqrt(E) with causal mask.
Within a softmax row i the w.q_i and b terms are constant across j and
cancel, so

    weights[i,j] = g_j / sum_{j'<=i} g_j',   g_j = exp(-w.k_j / sqrt(E))
    out[i,:]     = (sum_{j<=i} g_j V[j,:]) / (sum_{j<=i} g_j)

i.e. a causal cumulative weighted average of V -- O(S*E) per (b,h) --
and the output does not depend on queries at all.

Device kernel per core (4 of the 32 (b,h) pairs), all fp16 IO:

  s = 128*k + (127 - row): row-REVERSED within each 128-block.
  Prefix = lower-triangular matmul (within block) + per-block carries.

  Layouts: kt[row, k, e] (e innermost, for the E-reduction);
  v/wg [row, e, k] (k innermost) so the g broadcast rides a middle dim
  and the v*g multiply runs in DVE 2x mode; PSUM/cw/out [row, k, e] so
  the PSUM->SBUF drains are contiguous copies (strided 2-byte writes
  measured ~4x slower on HW); host untransposes the output.

  Per pair: sk = reduce_e(kt) via two fp16 halving adds + fp32 reduce
  (DVE); g = exp(sk) (ACT); wg = v*g (DVE 2x); two 512-col fp16 prefix
  matmuls (lower-tri) into 2 PSUM banks; block totals bs = colsum(wg)
  on GPSIMD (partition-reduce, SBUF-only -- avoids any PSUM readback);
  SBUF->SBUF scatter puts bs on 16 partitions; rm = strict-lower mask
  * bs (DVE 2x); carry matmuls (ones16 @ rm) accumulate into the same
  banks; ACT drains PSUM -> fp16 cw (contiguous); out = cw * (1/den)
  (DVE 1x, r broadcast is on the last dim).

  Denominator: separate tiny pipeline over g (one PSUM bank for all 4
  pairs): prefix matmul, GPSIMD colsum of G, scatter, masked carry,
  reciprocal -> r[128, pair, k].

  All mask/triangle constants are baked on the host and arrive in one
  DMA (GPSIMD affine-select setup measured ~3.5us and serialized
  against compute).
"""

import numpy as np

B, L, S, H, E = 4, 2048, 2048, 8, 64
NCORES = 8
PAIRS = (B * H) // NCORES  # 4 (b,h) pairs per core
NBLK = S // 128  # 16 blocks of 128 positions
DUOS = PAIRS // 2  # pairs processed two at a time
SCALE = np.float32(1.0 / np.sqrt(np.float32(E)))

# consts tensor column map (fp16, [128, CW] with sub-tiles at partition 0)
CW_TRI = 0  # triL [128, 0:128]
CW_ONES = 128  # ones16 [16, 128:256]
CW_MASK3 = 256  # mask3 [16, 256:1280]  (k', k, e) strict-lower
CW_MASK3D = 1280  # mask3d [16, 1280:1312]  (k', j, k) strict-lower
CWID = 1312

TRACE = False
LAST_RESULTS = None

_compiled = None


def _consts_host() -> np.ndarray:
    c = np.zeros((128, CWID), dtype=np.float16)
    # triL[p', p] = 1 iff p' >= p (row-reversed prefix)
    c[:, CW_TRI : CW_TRI + 128] = np.tril(np.ones((128, 128), np.float16))
    c[:16, CW_ONES : CW_ONES + 128] = 1.0
    kp = np.arange(16)
    m3 = (kp[:, None] < kp[None, :]).astype(np.float16)  # [k', k]
    c[:16, CW_MASK3 : CW_MASK3 + NBLK * E] = np.repeat(
        m3, E, axis=1
    )  # (k, e) e-inner
    c[:16, CW_MASK3D : CW_MASK3D + 2 * NBLK] = np.concatenate(
        [m3, m3], axis=1
    )  # (j, k) k-inner
    return c


def _build():
    from concourse import bacc
    import concourse.mybir as mybir
    import concourse.tile as tile

    f16 = mybir.dt.float16
    f32 = mybir.dt.float32
    nc = bacc.Bacc("TRN2", target_bir_lowering=False, debug=False)

    # (duo, row, pair-in-duo, ...) fp16; 4KB contiguous per partition line
    ktin = nc.dram_tensor("ktin", [DUOS, 128, 2, NBLK, E], f16, kind="ExternalInput")
    vin = nc.dram_tensor("vin", [DUOS, 128, 2, E, NBLK], f16, kind="ExternalInput")
    cin = nc.dram_tensor("cin", [128, CWID], f16, kind="ExternalInput")
    outT = nc.dram_tensor("outT", [DUOS, 128, 2, NBLK, E], f16, kind="ExternalOutput")

    with tile.TileContext(nc) as tc:
        with (
            tc.tile_pool(name="const", bufs=1) as cpool,
            tc.tile_pool(name="ktp", bufs=2) as ktp,
            tc.tile_pool(name="vp", bufs=2) as vp,
            tc.tile_pool(name="s1p", bufs=2) as s1p,
            tc.tile_pool(name="s2p", bufs=2) as s2p,
            tc.tile_pool(name="skp", bufs=2) as skp,
            tc.tile_pool(name="wgp", bufs=2) as wgp,
            tc.tile_pool(name="bs1p", bufs=2) as bs1p,
            tc.tile_pool(name="bsp", bufs=2) as bsp,
            tc.tile_pool(name="rmp", bufs=2) as rmp,
            tc.tile_pool(name="dbs1p", bufs=2) as dbs1p,
            tc.tile_pool(name="dbsp", bufs=2) as dbsp,
            tc.tile_pool(name="drmp", bufs=2) as drmp,
            tc.tile_pool(name="cwp", bufs=2) as cwp,
            tc.tile_pool(name="otp", bufs=2) as otp,
            tc.tile_pool(name="psp", bufs=3, space="PSUM") as psp,
            tc.tile_pool(name="dpsp", bufs=1, space="PSUM") as dpsp,
        ):
            consts = cpool.tile([128, CWID], f16)
            nc.sync.dma_start(out=consts[:], in_=cin[:])
            triL = consts[:, CW_TRI : CW_TRI + 128]
            ones16 = consts[0:16, CW_ONES : CW_ONES + 128]
            mask3 = consts[0:16, CW_MASK3 : CW_MASK3 + NBLK * E].rearrange(
                "p (k e) -> p k e", k=NBLK
            )
            mask3d = consts[0:16, CW_MASK3D : CW_MASK3D + 2 * NBLK].rearrange(
                "p (j k) -> p j k", j=2
            )

            G = cpool.tile([128, PAIRS, NBLK], f16)
            r = cpool.tile([128, PAIRS, NBLK], f16)
            den = dpsp.tile([128, PAIRS, NBLK], f32)  # one PSUM bank

            # --- stream all inputs up front (SP ring), duo 0 first ---
            kts, vs = [], []
            for d in range(DUOS):
                kt = ktp.tile([128, 2, NBLK, E], f16, tag="kt")
                v = vp.tile([128, 2, E, NBLK], f16, tag="v")
                nc.sync.dma_start(out=kt[:], in_=ktin[d])
                nc.sync.dma_start(out=v[:], in_=vin[d])
                kts.append(kt)
                vs.append(v)

            pss = {}
            for d in range(DUOS):
                kt, v = kts[d], vs[d]
                # sk = sum_e kt  (two fp16 halving adds, then fp32 reduce)
                s1 = s1p.tile([128, 2, NBLK, 32], f16, tag="s1")
                nc.vector.tensor_tensor(
                    out=s1[:], in0=kt[:, :, :, 0:32], in1=kt[:, :, :, 32:64],
                    op=mybir.AluOpType.add,
                )
                s2 = s2p.tile([128, 2, NBLK, 16], f16, tag="s2")
                nc.vector.tensor_tensor(
                    out=s2[:], in0=s1[:, :, :, 0:16], in1=s1[:, :, :, 16:32],
                    op=mybir.AluOpType.add,
                )
                sk = skp.tile([128, 2, NBLK], f32, tag="sk")
                nc.vector.tensor_reduce(
                    sk[:], s2[:], mybir.AxisListType.X, mybir.AluOpType.add
                )
                # g = exp(sk) -> fp16 (ACT)
                nc.scalar.activation(
                    G[:, 2 * d : 2 * d + 2, :], sk[:],
                    mybir.ActivationFunctionType.Exp,
                )
                # den prefix for this duo's two pairs (shared bank)
                nc.tensor.matmul(
                    den[:, 2 * d : 2 * d + 2, :], lhsT=triL,
                    rhs=G[:, 2 * d : 2 * d + 2, :],
                    start=True, stop=False, skip_group_check=True,
                )
                # wg = v * g  (g broadcast on middle dim -> 2x)
                wg = wgp.tile([128, 2, E, NBLK], f16, tag="wg")
                gb = (
                    G[:, 2 * d : 2 * d + 2, :]
                    .rearrange("p j (o k) -> p j o k", o=1)
                    .broadcast_to([128, 2, E, NBLK])
                )
                nc.vector.tensor_tensor(
                    out=wg[:], in0=v[:], in1=gb, op=mybir.AluOpType.mult
                )

                # block totals bs[k,e] = colsum(wg): GPSIMD partition-reduce
                # (SBUF-only), written (k,e)-contiguous for a cheap scatter
                bs1 = bs1p.tile([1, 2, NBLK, E], f16, tag="bs1")
                with nc.allow_low_precision("fp16 block sums feed fp16 carry"):
                    for j in range(2):
                        nc.gpsimd.tensor_reduce(
                            bs1[:, j],
                            wg[:, j].rearrange("p e k -> p k e"),
                            mybir.AxisListType.C,
                            mybir.AluOpType.add,
                        )
                # den colsums likewise (tiny), (k,j)-ordered for the scatter
                dbs1 = dbs1p.tile([1, NBLK, 2], f16, tag="dbs1")
                with nc.allow_low_precision("fp16 den sums feed fp16 carry"):
                    nc.gpsimd.tensor_reduce(
                        dbs1[:],
                        G[:, 2 * d : 2 * d + 2, :].rearrange("p j k -> p k j"),
                        mybir.AxisListType.C,
                        mybir.AluOpType.add,
                    )
                # SBUF->SBUF scatters onto 16 partitions (ACT ring)
                bs = bsp.tile([16, 2, E], f16, tag="bs")
                for j in range(2):
                    nc.scalar.dma_start(out=bs[:, j, :], in_=bs1[:, j])
                dbs = dbsp.tile([16, 2], f16, tag="dbs")
                nc.scalar.dma_start(out=dbs[:], in_=dbs1[:])

                # within-block prefix matmuls: PSUM (k, e), 512 cols per bank
                for j in range(2):
                    ps = psp.tile([128, NBLK, E], f32, tag="ps")  # 2 banks
                    rhs = wg[:, j].rearrange("p e k -> p k e")
                    nc.tensor.matmul(
                        ps[:, 0:8, :], lhsT=triL, rhs=rhs[:, 0:8, :],
                        start=True, stop=False, skip_group_check=True,
                    )
                    nc.tensor.matmul(
                        ps[:, 8:16, :], lhsT=triL, rhs=rhs[:, 8:16, :],
                        start=True, stop=False, skip_group_check=True,
                    )
                    pss[(d, j)] = ps

                # rm[k', j, k, e] = mask3[k',k,e] * bs[k',j,e]  (2x)
                rm = rmp.tile([16, 2, NBLK, E], f16, tag="rm")
                nc.vector.tensor_tensor(
                    out=rm[:],
                    in0=mask3.rearrange("p (o k) e -> p o k e", o=1).broadcast_to(
                        [16, 2, NBLK, E]
                    ),
                    in1=bs[:].rearrange("p j (o e) -> p j o e", o=1).broadcast_to(
                        [16, 2, NBLK, E]
                    ),
                    op=mybir.AluOpType.mult,
                )
                # den carry rm (tiny, 1x)
                drm = drmp.tile([16, 2, NBLK], f16, tag="drm")
                nc.vector.tensor_tensor(
                    out=drm[:],
                    in0=mask3d,
                    in1=dbs[:].rearrange("p (j o) -> p j o", o=1).broadcast_to(
                        [16, 2, NBLK]
                    ),
                    op=mybir.AluOpType.mult,
                )

                # carry matmuls accumulate into the same banks
                for j in range(2):
                    nc.tensor.matmul(
                        pss[(d, j)][:, 0:8, :], lhsT=ones16,
                        rhs=rm[:, j, 0:8, :],
                        start=False, stop=True, skip_group_check=True,
                    )
                    nc.tensor.matmul(
                        pss[(d, j)][:, 8:16, :], lhsT=ones16,
                        rhs=rm[:, j, 8:16, :],
                        start=False, stop=True, skip_group_check=True,
                    )
                nc.tensor.matmul(
                    den[:, 2 * d : 2 * d + 2, :], lhsT=ones16, rhs=drm[:],
                    start=False, stop=True, skip_group_check=True,
                )
                with nc.allow_low_precision("fp16 reciprocal feeds fp16 output"):
                    nc.vector.reciprocal(
                        r[:, 2 * d : 2 * d + 2, :], den[:, 2 * d : 2 * d + 2, :]
                    )

                # ACT drains PSUM -> fp16 cw, contiguous (k,e)
                cw = cwp.tile([128, 2, NBLK, E], f16, tag="cw")
                for j in range(2):
                    nc.scalar.copy(cw[:, j], pss[(d, j)][:])

                # out = cw * r  (r broadcast on last dim: 1x)
                ot = otp.tile([128, 2, NBLK, E], f16, tag="ot")
                rb = (
                    r[:, 2 * d : 2 * d + 2, :]
                    .rearrange("p j (k o) -> p j k o", o=1)
                    .broadcast_to([128, 2, NBLK, E])
                )
                nc.vector.tensor_tensor(
                    out=ot[:], in0=cw[:], in1=rb, op=mybir.AluOpType.mult
                )
                nc.sync.dma_start(out=outT[d], in_=ot[:])

    nc.compile()
    return nc


def _get_compiled():
    global _compiled
    if _compiled is None:
        _compiled = _build()
    return _compiled


def prep_inputs(keys: np.ndarray, values: np.ndarray, w_score: np.ndarray):
    """Host-side reshard: returns in_maps (list of 8 dicts)."""
    keys = np.asarray(keys, dtype=np.float32)
    values = np.asarray(values, dtype=np.float32)
    w = np.asarray(w_score, dtype=np.float32)

    # [B,S,H,E] -> [B*H, NBLK, 128, E], rows reversed within each block
    kt = keys.transpose(0, 2, 1, 3).reshape(B * H, NBLK, 128, E)[:, :, ::-1, :]
    kt = (kt * (-SCALE * w)).astype(np.float16)
    # -> [B*H, 128, NBLK, E]  (row, k, e)
    kt = kt.transpose(0, 2, 1, 3)

    v = values.transpose(0, 2, 1, 3).reshape(B * H, NBLK, 128, E)[:, :, ::-1, :]
    v = v.astype(np.float16).transpose(0, 2, 3, 1)  # [B*H, 128, E, NBLK]

    consts = _consts_host()
    in_maps = []
    for c in range(NCORES):
        sl = slice(PAIRS * c, PAIRS * (c + 1))
        ktc = kt[sl]  # [4, 128, NBLK, E]
        vc = v[sl]  # [4, 128, E, NBLK]
        # [duo, row, pair-in-duo, ...]
        ktc = np.ascontiguousarray(
            ktc.reshape(DUOS, 2, 128, NBLK, E).transpose(0, 2, 1, 3, 4)
        )
        vc = np.ascontiguousarray(
            vc.reshape(DUOS, 2, 128, E, NBLK).transpose(0, 2, 1, 3, 4)
        )
        in_maps.append({"ktin": ktc, "vin": vc, "cin": consts})
    return in_maps


def assemble_output(results) -> np.ndarray:
    # results[c]["outT"]: [DUOS, 128, 2, NBLK, E]; s = 128*k + (127-row)
    arr = np.stack([np.asarray(r["outT"]) for r in results])  # [8,D,128,2,K,E]
    arr = arr.transpose(0, 1, 3, 2, 4, 5).reshape(B * H, 128, NBLK, E)
    arr = arr.transpose(0, 2, 1, 3)[:, :, ::-1, :]  # [BH, k, row_rev, E]
    arr = arr.reshape(B, H, L, E).transpose(0, 2, 1, 3).astype(np.float32)
    return np.ascontiguousarray(arr)


def kernel(queries=None, keys=None, values=None, w_score=None, b_score=None, attn_mask=None, **_):
    global LAST_RESULTS
    from concourse.bass_utils import run_bass_kernel_spmd

    nc = _get_compiled()
    in_maps = prep_inputs(keys, values, w_score)
    res = run_bass_kernel_spmd(nc, in_maps, core_ids=list(range(NCORES)), trace=TRACE)
    LAST_RESULTS = res
    return assemble_output(res.results)


# revision 11
# speedup vs baseline: 10.3543x; 10.3543x over previous
"""MinusAttention kernel for Trainium2 (8 NeuronCores, Bass/Tile).

Math: score[i,j] = (w.q_i - w.k_j + b) / sqrt(E) with causal mask.
Within a softmax row i the w.q_i and b terms are constant across j and
cancel, so

    weights[i,j] = g_j / sum_{j'<=i} g_j',   g_j = exp(-w.k_j / sqrt(E))
    out[i,:]     = (sum_{j<=i} g_j V[j,:]) / (sum_{j<=i} g_j)

i.e. a causal cumulative weighted average of V -- O(S*E) per (b,h) --
and the output does not depend on queries at all.

Device kernel per core (4 of the 32 (b,h) pairs), all fp16 IO:

  s = 128*k + (127 - row): row-REVERSED within each 128-block.
  Prefix = lower-triangular matmul (within block) + per-block carries.

  Layouts: kt[row, k, e] (e innermost, for the E-reduction);
  v/wg [row, e, k] (k innermost) so the g broadcast rides a middle dim
  and the v*g multiply runs in DVE 2x mode; PSUM/cw/out [row, k, e] so
  the PSUM->SBUF drains are contiguous copies (strided 2-byte writes
  measured ~4x slower on HW); host untransposes the output.

  Per pair: sk = reduce_e(kt) via two fp16 halving adds + fp32 reduce
  (DVE); g = exp(sk) (ACT); wg = v*g (DVE 2x); two 512-col fp16 prefix
  matmuls (lower-tri) into 2 PSUM banks; block totals bs = colsum(wg)
  on GPSIMD (partition-reduce, SBUF-only -- avoids any PSUM readback);
  SBUF->SBUF scatter puts bs on 16 partitions; rm = strict-lower mask
  * bs (DVE 2x); carry matmuls (ones16 @ rm) accumulate into the same
  banks; ACT drains PSUM -> fp16 cw (contiguous); out = cw * (1/den)
  (DVE 1x, r broadcast is on the last dim).

  Denominator: separate tiny pipeline over g (one PSUM bank for all 4
  pairs): prefix matmul, GPSIMD colsum of G, scatter, masked carry,
  reciprocal -> r[128, pair, k].

  All mask/triangle constants are baked on the host and arrive in one
  DMA (GPSIMD affine-select setup measured ~3.5us and serialized
  against compute).
"""

import numpy as np

B, L, S, H, E = 4, 2048, 2048, 8, 64
NCORES = 8
PAIRS = (B * H) // NCORES  # 4 (b,h) pairs per core
NBLK = S // 128  # 16 blocks of 128 positions
DUOS = PAIRS // 2  # pairs processed two at a time
SCALE = np.float32(1.0 / np.sqrt(np.float32(E)))

# consts tensor column map (fp16, [128, CW] with sub-tiles at partition 0)
CW_TRI = 0  # triL [128, 0:128]
CW_ONES = 128  # ones16 [16, 128:256]
CW_MASK3 = 256  # mask3 [16, 256:1280]  (k', k, e) strict-lower
CW_MASK3D = 1280  # mask3d [16, 1280:1312]  (k', j, k) strict-lower
CWID = 1312

TRACE = False
LAST_RESULTS = None

_compiled = None


def _consts_host() -> np.ndarray:
    c = np.zeros((128, CWID), dtype=np.float16)
    # triL[p', p] = 1 iff p' >= p (row-reversed prefix)
    c[:, CW_TRI : CW_TRI + 128] = np.tril(np.ones((128, 128), np.float16))
    c[:16, CW_ONES : CW_ONES + 128] = 1.0
    kp = np.arange(16)
    m3 = (kp[:, None] < kp[None, :]).astype(np.float16)  # [k', k]
    c[:16, CW_MASK3 : CW_MASK3 + NBLK * E] = np.repeat(
        m3, E, axis=1
    )  # (k, e) e-inner
    c[:16, CW_MASK3D : CW_MASK3D + 2 * NBLK] = np.concatenate(
        [m3, m3], axis=1
    )  # (j, k) k-inner
    return c


def _build():
    from concourse import bacc
    import concourse.mybir as mybir
    import concourse.tile as tile

    f16 = mybir.dt.float16
    f32 = mybir.dt.float32
    nc = bacc.Bacc("TRN2", target_bir_lowering=False, debug=False)

    # (duo, row, pair-in-duo, ...) fp16; 4KB contiguous per partition line
    ktin = nc.dram_tensor("ktin", [DUOS, 128, 2, NBLK, E], f16, kind="ExternalInput")
    vin = nc.dram_tensor("vin", [DUOS, 128, 2, E, NBLK], f16, kind="ExternalInput")
    cin = nc.dram_tensor("cin", [128, CWID], f16, kind="ExternalInput")
    outT = nc.dram_tensor("outT", [DUOS, 128, 2, NBLK, E], f16, kind="ExternalOutput")

    with tile.TileContext(nc) as tc:
        with (
            tc.tile_pool(name="const", bufs=1) as cpool,
            tc.tile_pool(name="ktp", bufs=2) as ktp,
            tc.tile_pool(name="vp", bufs=2) as vp,
            tc.tile_pool(name="s1p", bufs=2) as s1p,
            tc.tile_pool(name="s2p", bufs=2) as s2p,
            tc.tile_pool(name="skp", bufs=2) as skp,
            tc.tile_pool(name="wgp", bufs=2) as wgp,
            tc.tile_pool(name="bs1p", bufs=2) as bs1p,
            tc.tile_pool(name="bsp", bufs=2) as bsp,
            tc.tile_pool(name="rmp", bufs=2) as rmp,
            tc.tile_pool(name="dbs1p", bufs=2) as dbs1p,
            tc.tile_pool(name="dbsp", bufs=2) as dbsp,
            tc.tile_pool(name="drmp", bufs=2) as drmp,
            tc.tile_pool(name="cwp", bufs=2) as cwp,
            tc.tile_pool(name="otp", bufs=2) as otp,
            tc.tile_pool(name="psp", bufs=3, space="PSUM") as psp,
            tc.tile_pool(name="dpsp", bufs=1, space="PSUM") as dpsp,
        ):
            consts = cpool.tile([128, CWID], f16)
            nc.sync.dma_start(out=consts[:], in_=cin[:])
            triL = consts[:, CW_TRI : CW_TRI + 128]
            ones16 = consts[0:16, CW_ONES : CW_ONES + 128]
            mask3 = consts[0:16, CW_MASK3 : CW_MASK3 + NBLK * E].rearrange(
                "p (k e) -> p k e", k=NBLK
            )
            mask3d = consts[0:16, CW_MASK3D : CW_MASK3D + 2 * NBLK].rearrange(
                "p (j k) -> p j k", j=2
            )

            G = cpool.tile([128, PAIRS, NBLK], f16)
            r = cpool.tile([128, PAIRS, NBLK], f16)
            den = dpsp.tile([128, PAIRS, NBLK], f32)  # one PSUM bank

            # --- stream all inputs up front (SP ring), duo 0 first ---
            kts, vs = [], []
            for d in range(DUOS):
                kt = ktp.tile([128, 2, NBLK, E], f16, tag="kt")
                v = vp.tile([128, 2, E, NBLK], f16, tag="v")
                nc.sync.dma_start(out=kt[:], in_=ktin[d])
                nc.sync.dma_start(out=v[:], in_=vin[d])
                kts.append(kt)
                vs.append(v)

            pss = {}
            for d in range(DUOS):
                kt, v = kts[d], vs[d]
                # sk = sum_e kt  (two fp16 halving adds, then fp32 reduce)
                s1 = s1p.tile([128, 2, NBLK, 32], f16, tag="s1")
                nc.vector.tensor_tensor(
                    out=s1[:], in0=kt[:, :, :, 0:32], in1=kt[:, :, :, 32:64],
                    op=mybir.AluOpType.add,
                )
                s2 = s2p.tile([128, 2, NBLK, 16], f16, tag="s2")
                nc.vector.tensor_tensor(
                    out=s2[:], in0=s1[:, :, :, 0:16], in1=s1[:, :, :, 16:32],
                    op=mybir.AluOpType.add,
                )
                sk = skp.tile([128, 2, NBLK], f32, tag="sk")
                nc.vector.tensor_reduce(
                    sk[:], s2[:], mybir.AxisListType.X, mybir.AluOpType.add
                )
                # g = exp(sk) -> fp16 (ACT)
                nc.scalar.activation(
                    G[:, 2 * d : 2 * d + 2, :], sk[:],
                    mybir.ActivationFunctionType.Exp,
                )
                # den prefix for this duo's two pairs (shared bank)
                nc.tensor.matmul(
                    den[:, 2 * d : 2 * d + 2, :], lhsT=triL,
                    rhs=G[:, 2 * d : 2 * d + 2, :],
                    start=True, stop=False, skip_group_check=True,
                )
                # wg = v * g  (g broadcast on middle dim -> 2x)
                wg = wgp.tile([128, 2, E, NBLK], f16, tag="wg")
                gb = (
                    G[:, 2 * d : 2 * d + 2, :]
                    .rearrange("p j (o k) -> p j o k", o=1)
                    .broadcast_to([128, 2, E, NBLK])
                )
                nc.vector.tensor_tensor(
                    out=wg[:], in0=v[:], in1=gb, op=mybir.AluOpType.mult
                )

                # within-block prefix matmuls: PSUM (k, e), 512 cols per bank
                for j in range(2):
                    ps = psp.tile([128, NBLK, E], f32, tag="ps")  # 2 banks
                    rhs = wg[:, j].rearrange("p e k -> p k e")
                    nc.tensor.matmul(
                        ps[:, 0:8, :], lhsT=triL, rhs=rhs[:, 0:8, :],
                        start=True, stop=False, skip_group_check=True,
                    )
                    nc.tensor.matmul(
                        ps[:, 8:16, :], lhsT=triL, rhs=rhs[:, 8:16, :],
                        start=True, stop=False, skip_group_check=True,
                    )
                    pss[(d, j)] = ps

                # block totals = PSUM row 0 (reversed rows): ACT extracts
                # them (contiguous (k,e) copy), SBUF->SBUF scatter puts them
                # on 16 partitions (SP ring)
                bs1 = bs1p.tile([1, 2, NBLK, E], f16, tag="bs1")
                for j in range(2):
                    nc.scalar.copy(bs1[:, j], pss[(d, j)][0:1, :, :])
                # den totals: tiny ACT extract, written (k,j)-ordered
                dbs1 = dbs1p.tile([1, NBLK, 2], f16, tag="dbs1")
                nc.scalar.copy(
                    dbs1[:].rearrange("p k j -> p j k"),
                    den[0:1, 2 * d : 2 * d + 2, :],
                )
                bs = bsp.tile([16, 2, E], f16, tag="bs")
                for j in range(2):
                    nc.sync.dma_start(out=bs[:, j, :], in_=bs1[:, j])
                dbs = dbsp.tile([16, 2], f16, tag="dbs")
                nc.gpsimd.dma_start(out=dbs[:], in_=dbs1[:])

                # rm[k', j, k, e] = mask3[k',k,e] * bs[k',j,e]  (2x)
                rm = rmp.tile([16, 2, NBLK, E], f16, tag="rm")
                nc.vector.tensor_tensor(
                    out=rm[:],
                    in0=mask3.rearrange("p (o k) e -> p o k e", o=1).broadcast_to(
                        [16, 2, NBLK, E]
                    ),
                    in1=bs[:].rearrange("p j (o e) -> p j o e", o=1).broadcast_to(
                        [16, 2, NBLK, E]
                    ),
                    op=mybir.AluOpType.mult,
                )
                # den carry rm (tiny, 1x)
                drm = drmp.tile([16, 2, NBLK], f16, tag="drm")
                nc.vector.tensor_tensor(
                    out=drm[:],
                    in0=mask3d,
                    in1=dbs[:].rearrange("p (j o) -> p j o", o=1).broadcast_to(
                        [16, 2, NBLK]
                    ),
                    op=mybir.AluOpType.mult,
                )

                # carry matmuls accumulate into the same banks
                for j in range(2):
                    nc.tensor.matmul(
                        pss[(d, j)][:, 0:8, :], lhsT=ones16,
                        rhs=rm[:, j, 0:8, :],
                        start=False, stop=True, skip_group_check=True,
                    )
                    nc.tensor.matmul(
                        pss[(d, j)][:, 8:16, :], lhsT=ones16,
                        rhs=rm[:, j, 8:16, :],
                        start=False, stop=True, skip_group_check=True,
                    )
                nc.tensor.matmul(
                    den[:, 2 * d : 2 * d + 2, :], lhsT=ones16, rhs=drm[:],
                    start=False, stop=True, skip_group_check=True,
                )
                with nc.allow_low_precision("fp16 reciprocal feeds fp16 output"):
                    nc.vector.reciprocal(
                        r[:, 2 * d : 2 * d + 2, :], den[:, 2 * d : 2 * d + 2, :]
                    )

                # ACT drains PSUM -> fp16 cw, contiguous (k,e)
                cw = cwp.tile([128, 2, NBLK, E], f16, tag="cw")
                for j in range(2):
                    nc.scalar.copy(cw[:, j], pss[(d, j)][:])

                # out = cw * r  (r broadcast on last dim: 1x); per-pair so
                # the tail pipelines; duo 0 on GPSIMD, duo 1 on DVE
                ot = otp.tile([128, 2, NBLK, E], f16, tag="ot")
                for j in range(2):
                    rb = (
                        r[:, 2 * d + j, :]
                        .rearrange("p (k o) -> p k o", o=1)
                        .broadcast_to([128, NBLK, E])
                    )
                    eng = nc.gpsimd if d == 0 else nc.vector
                    eng.tensor_tensor(
                        out=ot[:, j], in0=cw[:, j], in1=rb,
                        op=mybir.AluOpType.mult,
                    )
                    nc.sync.dma_start(out=outT[d, :, j], in_=ot[:, j])

    nc.compile()
    return nc


def _get_compiled():
    global _compiled
    if _compiled is None:
        _compiled = _build()
    return _compiled


def prep_inputs(keys: np.ndarray, values: np.ndarray, w_score: np.ndarray):
    """Host-side reshard: returns in_maps (list of 8 dicts)."""
    keys = np.asarray(keys, dtype=np.float32)
    values = np.asarray(values, dtype=np.float32)
    w = np.asarray(w_score, dtype=np.float32)

    # [B,S,H,E] -> [B*H, NBLK, 128, E], rows reversed within each block
    kt = keys.transpose(0, 2, 1, 3).reshape(B * H, NBLK, 128, E)[:, :, ::-1, :]
    kt = (kt * (-SCALE * w)).astype(np.float16)
    # -> [B*H, 128, NBLK, E]  (row, k, e)
    kt = kt.transpose(0, 2, 1, 3)

    v = values.transpose(0, 2, 1, 3).reshape(B * H, NBLK, 128, E)[:, :, ::-1, :]
    v = v.astype(np.float16).transpose(0, 2, 3, 1)  # [B*H, 128, E, NBLK]

    consts = _consts_host()
    in_maps = []
    for c in range(NCORES):
        sl = slice(PAIRS * c, PAIRS * (c + 1))
        ktc = kt[sl]  # [4, 128, NBLK, E]
        vc = v[sl]  # [4, 128, E, NBLK]
        # [duo, row, pair-in-duo, ...]
        ktc = np.ascontiguousarray(
            ktc.reshape(DUOS, 2, 128, NBLK, E).transpose(0, 2, 1, 3, 4)
        )
        vc = np.ascontiguousarray(
            vc.reshape(DUOS, 2, 128, E, NBLK).transpose(0, 2, 1, 3, 4)
        )
        in_maps.append({"ktin": ktc, "vin": vc, "cin": consts})
    return in_maps


def assemble_output(results) -> np.ndarray:
    # results[c]["outT"]: [DUOS, 128, 2, NBLK, E]; s = 128*k + (127-row)
    arr = np.stack([np.asarray(r["outT"]) for r in results])  # [8,D,128,2,K,E]
    arr = arr.transpose(0, 1, 3, 2, 4, 5).reshape(B * H, 128, NBLK, E)
    arr = arr.transpose(0, 2, 1, 3)[:, :, ::-1, :]  # [BH, k, row_rev, E]
    arr = arr.reshape(B, H, L, E).transpose(0, 2, 1, 3).astype(np.float32)
    return np.ascontiguousarray(arr)


def kernel(queries=None, keys=None, values=None, w_score=None, b_score=None, attn_mask=None, **_):
    global LAST_RESULTS
    from concourse.bass_utils import run_bass_kernel_spmd

    nc = _get_compiled()
    in_maps = prep_inputs(keys, values, w_score)
    res = run_bass_kernel_spmd(nc, in_maps, core_ids=list(range(NCORES)), trace=TRACE)
    LAST_RESULTS = res
    return assemble_output(res.results)


# revision 15
# speedup vs baseline: 12.8340x; 1.2395x over previous
"""MinusAttention kernel for Trainium2 (8 NeuronCores, Bass/Tile).

Math: score[i,j] = (w.q_i - w.k_j + b) / sqrt(E) with causal mask.
Within a softmax row i the w.q_i and b terms are constant across j and
cancel, so

    weights[i,j] = g_j / sum_{j'<=i} g_j',   g_j = exp(-w.k_j / sqrt(E))
    out[i,:]     = (sum_{j<=i} g_j V[j,:]) / (sum_{j<=i} g_j)

i.e. a causal cumulative weighted average of V -- O(S*E) per (b,h) --
and the output does not depend on queries at all.

Device kernel per core (4 of the 32 (b,h) pairs), all fp16 IO:

  s = 128*k + (127 - row): row-REVERSED within each 128-block.
  Prefix = lower-triangular matmul (within block) + per-block carries.

  Layouts: kt[row, k, e] (e innermost, for the E-reduction);
  v/wg [row, e, k] (k innermost) so the g broadcast rides a middle dim
  and the v*g multiply runs in DVE 2x mode; PSUM/cw/out [row, k, e] so
  the PSUM->SBUF drains are contiguous copies (strided 2-byte writes
  measured ~4x slower on HW); host untransposes the output.

  Per pair: sk = reduce_e(kt) via two fp16 halving adds + fp32 reduce
  (DVE); g = exp(sk) (ACT); wg = v*g (DVE 2x); two 512-col fp16 prefix
  matmuls (lower-tri) into 2 PSUM banks; block totals bs = colsum(wg)
  on GPSIMD (partition-reduce, SBUF-only -- avoids any PSUM readback);
  SBUF->SBUF scatter puts bs on 16 partitions; rm = strict-lower mask
  * bs (DVE 2x); carry matmuls (ones16 @ rm) accumulate into the same
  banks; ACT drains PSUM -> fp16 cw (contiguous); out = cw * (1/den)
  (DVE 1x, r broadcast is on the last dim).

  Denominator: separate tiny pipeline over g (one PSUM bank for all 4
  pairs): prefix matmul, GPSIMD colsum of G, scatter, masked carry,
  reciprocal -> r[128, pair, k].

  All mask/triangle constants are baked on the host and arrive in one
  DMA (GPSIMD affine-select setup measured ~3.5us and serialized
  against compute).
"""

import numpy as np

B, L, S, H, E = 4, 2048, 2048, 8, 64
NCORES = 8
PAIRS = (B * H) // NCORES  # 4 (b,h) pairs per core
NBLK = S // 128  # 16 blocks of 128 positions
DUOS = PAIRS // 2  # pairs processed two at a time
SCALE = np.float32(1.0 / np.sqrt(np.float32(E)))

# consts tensor column map (fp16, [128, CW] with sub-tiles at partition 0)
CW_TRI = 0  # triL [128, 0:128]
CW_ONES = 128  # ones16 [16, 128:256]
CW_MASK3 = 256  # mask3 [16, 256:1280]  (k', k, e) strict-lower
CW_MASK3D = 1280  # mask3d [16, 1280:1312]  (k', j, k) strict-lower
CWID = 1312

TRACE = False
LAST_RESULTS = None

_compiled = None


def _consts_host() -> np.ndarray:
    c = np.zeros((128, CWID), dtype=np.float16)
    # triL[p', p] = 1 iff p' >= p (row-reversed prefix)
    c[:, CW_TRI : CW_TRI + 128] = np.tril(np.ones((128, 128), np.float16))
    c[:16, CW_ONES : CW_ONES + 128] = 1.0
    kp = np.arange(16)
    m3 = (kp[:, None] < kp[None, :]).astype(np.float16)  # [k', k]
    c[:16, CW_MASK3 : CW_MASK3 + NBLK * E] = np.repeat(
        m3, E, axis=1
    )  # (k, e) e-inner
    c[:16, CW_MASK3D : CW_MASK3D + 2 * NBLK] = np.concatenate(
        [m3, m3], axis=1
    )  # (j, k) k-inner
    return c


def _build():
    from concourse import bacc
    import concourse.mybir as mybir
    import concourse.tile as tile

    f16 = mybir.dt.float16
    f32 = mybir.dt.float32
    nc = bacc.Bacc("TRN2", target_bir_lowering=False, debug=False)

    # (duo, row, pair-in-duo, ...) fp16; 4KB contiguous per partition line
    ktin = nc.dram_tensor("ktin", [DUOS, 128, 2, NBLK, E], f16, kind="ExternalInput")
    vin = nc.dram_tensor("vin", [DUOS, 128, 2, E, NBLK], f16, kind="ExternalInput")
    cin = nc.dram_tensor("cin", [128, CWID], f16, kind="ExternalInput")
    outT = nc.dram_tensor("outT", [DUOS, 128, 2, NBLK, E], f16, kind="ExternalOutput")

    with tile.TileContext(nc) as tc:
        with (
            tc.tile_pool(name="const", bufs=1) as cpool,
            tc.tile_pool(name="ktp", bufs=2) as ktp,
            tc.tile_pool(name="vp", bufs=2) as vp,
            tc.tile_pool(name="s1p", bufs=2) as s1p,
            tc.tile_pool(name="s2p", bufs=2) as s2p,
            tc.tile_pool(name="skp", bufs=2) as skp,
            tc.tile_pool(name="wgp", bufs=2) as wgp,
            tc.tile_pool(name="bs1p", bufs=2) as bs1p,
            tc.tile_pool(name="bsp", bufs=2) as bsp,
            tc.tile_pool(name="rmp", bufs=2) as rmp,
            tc.tile_pool(name="dbs1p", bufs=2) as dbs1p,
            tc.tile_pool(name="dbsp", bufs=2) as dbsp,
            tc.tile_pool(name="drmp", bufs=2) as drmp,
            tc.tile_pool(name="cwp", bufs=2) as cwp,
            tc.tile_pool(name="otp", bufs=2) as otp,
            tc.tile_pool(name="psp", bufs=3, space="PSUM") as psp,
            tc.tile_pool(name="dpsp", bufs=2, space="PSUM") as dpsp,
        ):
            consts = cpool.tile([128, CWID], f16)
            nc.sync.dma_start(out=consts[:], in_=cin[:])
            triL = consts[:, CW_TRI : CW_TRI + 128]
            ones16 = consts[0:16, CW_ONES : CW_ONES + 128]
            mask3 = consts[0:16, CW_MASK3 : CW_MASK3 + NBLK * E].rearrange(
                "p (k e) -> p k e", k=NBLK
            )
            mask3d = consts[0:16, CW_MASK3D : CW_MASK3D + 2 * NBLK].rearrange(
                "p (j k) -> p j k", j=2
            )

            G = cpool.tile([128, PAIRS, NBLK], f16)
            r = cpool.tile([128, PAIRS, NBLK], f16)
            # one PSUM bank per duo: a start=True matmul resets the whole
            # bank, so duos must not share one
            dens = [
                dpsp.tile([128, 2, NBLK], f32, tag="den", name=f"den{d}")
                for d in range(DUOS)
            ]

            # --- stream all inputs up front (SP ring), duo 0 first ---
            kts, vs = [], []
            for d in range(DUOS):
                kt = ktp.tile([128, 2, NBLK, E], f16, tag="kt")
                v = vp.tile([128, 2, E, NBLK], f16, tag="v")
                nc.sync.dma_start(out=kt[:], in_=ktin[d])
                nc.sync.dma_start(out=v[:], in_=vin[d])
                kts.append(kt)
                vs.append(v)

            pss = {}
            wgs, bs1s, bss, dbss, cws, ots = {}, {}, {}, {}, {}, {}

            # --- phase A: g-pipeline for both duos (DVE/ACT), den prefix ---
            for d in range(DUOS):
                kt, v = kts[d], vs[d]
                s1 = s1p.tile([128, 2, NBLK, 32], f16, tag="s1")
                nc.vector.tensor_tensor(
                    out=s1[:], in0=kt[:, :, :, 0:32], in1=kt[:, :, :, 32:64],
                    op=mybir.AluOpType.add,
                )
                s2 = s2p.tile([128, 2, NBLK, 16], f16, tag="s2")
                nc.vector.tensor_tensor(
                    out=s2[:], in0=s1[:, :, :, 0:16], in1=s1[:, :, :, 16:32],
                    op=mybir.AluOpType.add,
                )
                sk = skp.tile([128, 2, NBLK], f32, tag="sk")
                nc.vector.tensor_reduce(
                    sk[:], s2[:], mybir.AxisListType.X, mybir.AluOpType.add
                )
                nc.scalar.activation(
                    G[:, 2 * d : 2 * d + 2, :], sk[:],
                    mybir.ActivationFunctionType.Exp,
                )
                nc.tensor.matmul(
                    dens[d][:], lhsT=triL,
                    rhs=G[:, 2 * d : 2 * d + 2, :],
                    start=True, stop=False, skip_group_check=True,
                )

            # --- phase B: wg for both duos (DVE 2x) ---
            for d in range(DUOS):
                wg = wgp.tile([128, 2, E, NBLK], f16, tag="wg")
                gb = (
                    G[:, 2 * d : 2 * d + 2, :]
                    .rearrange("p j (o k) -> p j o k", o=1)
                    .broadcast_to([128, 2, E, NBLK])
                )
                nc.vector.tensor_tensor(
                    out=wg[:], in0=vs[d][:], in1=gb, op=mybir.AluOpType.mult
                )
                wgs[d] = wg

            # --- phase C: prefix matmuls for the first 3 pairs (PE ramps;
            # the 4th pair's PSUM tile rotates onto pair 0's banks, so its
            # matmuls are emitted only after pair 0's drain) ---
            PAIRLIST = [(0, 0), (0, 1), (1, 0), (1, 1)]

            def emit_pmm(d, j):
                ps = psp.tile([128, NBLK, E], f32, tag="ps", name=f"ps{d}{j}")
                rhs = wgs[d][:, j].rearrange("p e k -> p k e")
                nc.tensor.matmul(
                    ps[:, 0:8, :], lhsT=triL, rhs=rhs[:, 0:8, :],
                    start=True, stop=False, skip_group_check=True,
                )
                nc.tensor.matmul(
                    ps[:, 8:16, :], lhsT=triL, rhs=rhs[:, 8:16, :],
                    start=True, stop=False, skip_group_check=True,
                )
                pss[(d, j)] = ps

            for d, j in PAIRLIST[:3]:
                emit_pmm(d, j)

            # --- phase D: den row-0 extracts + scatters (early, off SP) ---
            for d in range(DUOS):
                dbs1 = dbs1p.tile([1, NBLK, 2], f16, tag="dbs1")
                nc.scalar.copy(
                    dbs1[:].rearrange("p k j -> p j k"), dens[d][0:1]
                )
                dbs = dbsp.tile([16, 2], f16, tag="dbs")
                nc.gpsimd.dma_start(out=dbs[:], in_=dbs1[:])
                dbss[d] = dbs

            # --- phase F: den carries + reciprocal (small) ---
            for d in range(DUOS):
                drm = drmp.tile([16, 2, NBLK], f16, tag="drm")
                nc.vector.tensor_tensor(
                    out=drm[:],
                    in0=mask3d,
                    in1=dbss[d][:].rearrange("p (j o) -> p j o", o=1).broadcast_to(
                        [16, 2, NBLK]
                    ),
                    op=mybir.AluOpType.mult,
                )
                nc.tensor.matmul(
                    dens[d][:], lhsT=ones16, rhs=drm[:],
                    start=False, stop=True, skip_group_check=True,
                )
                with nc.allow_low_precision("fp16 reciprocal feeds fp16 output"):
                    nc.vector.reciprocal(
                        r[:, 2 * d : 2 * d + 2, :], dens[d][:]
                    )

            # --- phase G: per-pair back-ends, pipelined ---
            for d in range(DUOS):
                cws[d] = cwp.tile([128, 2, NBLK, E], f16, tag="cw", name=f"cw{d}")
                ots[d] = otp.tile([128, 2, NBLK, E], f16, tag="ot", name=f"ot{d}")

            def emit_extract_scatter(d, j):
                bs1 = bs1p.tile([1, NBLK, E], f16, tag="bs1", name=f"bs1_{d}{j}")
                nc.scalar.copy(bs1[:], pss[(d, j)][0:1, :, :])
                bs = bsp.tile([16, E], f16, tag="bs", name=f"bs{d}{j}")
                nc.sync.dma_start(out=bs[:], in_=bs1[:])
                bss[(d, j)] = bs

            def emit_backend(d, j):
                rm = rmp.tile([16, NBLK, E], f16, tag="rm", name=f"rm{d}{j}")
                nc.vector.tensor_tensor(
                    out=rm[:],
                    in0=mask3,
                    in1=bss[(d, j)][:].rearrange(
                        "p (o e) -> p o e", o=1
                    ).broadcast_to([16, NBLK, E]),
                    op=mybir.AluOpType.mult,
                )
                nc.tensor.matmul(
                    pss[(d, j)][:, 0:8, :], lhsT=ones16, rhs=rm[:, 0:8, :],
                    start=False, stop=True, skip_group_check=True,
                )
                nc.tensor.matmul(
                    pss[(d, j)][:, 8:16, :], lhsT=ones16, rhs=rm[:, 8:16, :],
                    start=False, stop=True, skip_group_check=True,
                )
                nc.scalar.copy(cws[d][:, j], pss[(d, j)][:])
                rb = (
                    r[:, 2 * d + j, :]
                    .rearrange("p (k o) -> p k o", o=1)
                    .broadcast_to([128, NBLK, E])
                )
                eng = nc.gpsimd if d == 0 else nc.vector
                eng.tensor_tensor(
                    out=ots[d][:, j], in0=cws[d][:, j], in1=rb,
                    op=mybir.AluOpType.mult,
                )

            for d, j in PAIRLIST[:3]:
                emit_extract_scatter(d, j)
            for d, j in PAIRLIST[:3]:
                emit_backend(d, j)
            # 4th pair: matmuls rotate onto pair 0's freed banks
            emit_pmm(1, 1)
            emit_extract_scatter(1, 1)
            emit_backend(1, 1)
            for d, j in PAIRLIST:
                nc.sync.dma_start(out=outT[d, :, j], in_=ots[d][:, j])

    nc.compile()
    return nc


def _get_compiled():
    global _compiled
    if _compiled is None:
        _compiled = _build()
    return _compiled


def prep_inputs(keys: np.ndarray, values: np.ndarray, w_score: np.ndarray):
    """Host-side reshard: returns in_maps (list of 8 dicts)."""
    keys = np.asarray(keys, dtype=np.float32)
    values = np.asarray(values, dtype=np.float32)
    w = np.asarray(w_score, dtype=np.float32)

    # [B,S,H,E] -> [B*H, NBLK, 128, E], rows reversed within each block
    kt = keys.transpose(0, 2, 1, 3).reshape(B * H, NBLK, 128, E)[:, :, ::-1, :]
    kt = (kt * (-SCALE * w)).astype(np.float16)
    # -> [B*H, 128, NBLK, E]  (row, k, e)
    kt = kt.transpose(0, 2, 1, 3)

    v = values.transpose(0, 2, 1, 3).reshape(B * H, NBLK, 128, E)[:, :, ::-1, :]
    v = v.astype(np.float16).transpose(0, 2, 3, 1)  # [B*H, 128, E, NBLK]

    consts = _consts_host()
    in_maps = []
    for c in range(NCORES):
        sl = slice(PAIRS * c, PAIRS * (c + 1))
        ktc = kt[sl]  # [4, 128, NBLK, E]
        vc = v[sl]  # [4, 128, E, NBLK]
        # [duo, row, pair-in-duo, ...]
        ktc = np.ascontiguousarray(
            ktc.reshape(DUOS, 2, 128, NBLK, E).transpose(0, 2, 1, 3, 4)
        )
        vc = np.ascontiguousarray(
            vc.reshape(DUOS, 2, 128, E, NBLK).transpose(0, 2, 1, 3, 4)
        )
        in_maps.append({"ktin": ktc, "vin": vc, "cin": consts})
    return in_maps


def assemble_output(results) -> np.ndarray:
    # results[c]["outT"]: [DUOS, 128, 2, NBLK, E]; s = 128*k + (127-row)
    arr = np.stack([np.asarray(r["outT"]) for r in results])  # [8,D,128,2,K,E]
    arr = arr.transpose(0, 1, 3, 2, 4, 5).reshape(B * H, 128, NBLK, E)
    arr = arr.transpose(0, 2, 1, 3)[:, :, ::-1, :]  # [BH, k, row_rev, E]
    arr = arr.reshape(B, H, L, E).transpose(0, 2, 1, 3).astype(np.float32)
    return np.ascontiguousarray(arr)


def kernel(queries=None, keys=None, values=None, w_score=None, b_score=None, attn_mask=None, **_):
    global LAST_RESULTS
    from concourse.bass_utils import run_bass_kernel_spmd

    nc = _get_compiled()
    in_maps = prep_inputs(keys, values, w_score)
    res = run_bass_kernel_spmd(nc, in_maps, core_ids=list(range(NCORES)), trace=TRACE)
    LAST_RESULTS = res
    return assemble_output(res.results)
